# revision 21
# baseline (speedup 1.0000x reference)
"""Adaptive average pooling [8,224,224,256] -> [8,7,7,256] on 8 TRN2 NeuronCores.

Strategy: data-parallel over batch (1 sample per core, no collectives).
Pooling windows are exact 32x32 blocks (224/7 = 32). Each sample is
transposed host-side to [H, C, W] and cast to bf16 (halves HBM traffic;
window sums accumulate in fp32 so only input quantization enters).

Per core:
  - partition dim = image row h, free dim = flattened (c, w); every DMA is
    fully contiguous (14 KiB per partition per tile).
  - stage 1 (reduce over the 32 h rows of each window): TensorE matmuls
    against a block matrix of 1/1024, contracting the partition dim. Output
    chunks are packed into PSUM partition quarters (4 chunks x 448 per
    quarter) so downstream ops use ~full lanes.
  - stage 2 (reduce over the 32 w positions): VectorE strided reduce
    directly from PSUM (only 1/32 of the volume reaches VectorE).
  - result is DMA'd out in the raw packed layout; host numpy unscrambles
    the 180 KB output.
"""

import ml_dtypes
import numpy as np

B, H, W, C = 8, 224, 224, 256
OH, OW = 7, 7
WIN = H // OH  # 32
CW = C * W  # 57344 elements per row, layout (c, w)
CG = 32  # channels per input tile
CHUNK = CG * W  # 7168 elements = 14 KiB (bf16) per partition
NCG = C // CG  # 8 channel groups
CPAIR = 2 * W  # 448: matmul rhs chunk = 2 channels x 224 w
YF = 4 * 2 * OW  # 56 floats of ybuf per channel group

_CACHE = {}


def _build():
    import concourse.bass as bass
    import concourse.mybir as mybir
    from concourse import bacc, tile

    f32 = mybir.dt.float32
    bf16 = mybir.dt.bfloat16
    nc = bacc.Bacc(
        "TRN2",
        target_bir_lowering=False,
        debug=False,
        enable_asserts=False,
        num_devices=B,
    )
    x = nc.dram_tensor("x", [H, CW], bf16, kind="ExternalInput").ap()
    mh = nc.dram_tensor("mh", [128, 32], bf16, kind="ExternalInput").ap()
    out = nc.dram_tensor("out", [2, 128, NCG * YF], f32, kind="ExternalOutput").ap()

    with tile.TileContext(nc) as tc:
        with (
            tc.tile_pool(name="consts", bufs=1) as cpool,
            tc.tile_pool(name="xin", bufs=6) as inpool,
            tc.tile_pool(name="ybuf", bufs=2) as ypool,
            tc.tile_pool(name="psum", bufs=2, space=bass.MemorySpace.PSUM) as ppool,
        ):
            mh_t = cpool.tile([128, 32], bf16)
            nc.sync.dma_start(mh_t[:], mh[:])
            for ht in range(2):  # rows 0..127, 128..223
                P = 128 if ht == 0 else H - 128
                I = P // WIN  # output rows this half: 4 or 3
                ybuf = ypool.tile([128, NCG * YF], f32)
                for cg in range(NCG):
                    t = inpool.tile([128, CHUNK], bf16)
                    nc.sync.dma_start(
                        t[:P, :],
                        x[ht * 128 : ht * 128 + P, cg * CHUNK : (cg + 1) * CHUNK],
                    )
                    ps = ppool.tile([128, 2048], f32)
                    # chunk m = channel pair; quarter q=m//4 rows 32q.., slot m%4
                    for m in range(16):
                        q, s = divmod(m, 4)
                        nc.tensor.matmul(
                            ps[32 * q : 32 * q + 32, 512 * s : 512 * s + CPAIR],
                            mh_t[:P, :32],
                            t[:P, m * CPAIR : (m + 1) * CPAIR],
                            start=True,
                            stop=True,
                            tile_position=(0, 32 * q),
                        )
                    # reduce w (unit stride innermost) straight out of PSUM:
                    # in [100][(s:4,x512)][(c2:2,x224)][(j:7,x32)][(w:32,x1)]
                    inap = ps[:128, :].rearrange("p (s f) -> p s f", s=4, f=512)[
                        :, :, :CPAIR
                    ].rearrange("p s (c j w) -> p s c j w", c=2, j=OW, w=WIN)
                    outap = ybuf[:128, cg * YF : (cg + 1) * YF].rearrange(
                        "p (s c j) -> p s c j", s=4, c=2, j=OW
                    )
                    nc.vector.tensor_reduce(
                        out=outap,
                        in_=inap,
                        axis=mybir.AxisListType.X,
                        op=mybir.AluOpType.add,
                    )
                # different queue than the input stream: a sync-engine wait
                # here would stall all queued ht1 input triggers behind it
                nc.scalar.dma_start(out[ht, :, :], ybuf[:, :])
    nc.compile()
    return nc


def _mh_matrix():
    # cols 0..3 select h-windows; cols 4..31 stay zero (they fill the unused
    # PSUM quarter rows with defined zeros at no extra TensorE cost)
    m = np.zeros((128, 32), dtype=ml_dtypes.bfloat16)
    for p in range(128):
        m[p, p // WIN] = 1.0 / (WIN * WIN)
    return m


def _unscramble(raw):
    """raw [2, 128, 448] packed -> y [7, 7, 256].

    raw[ht, 32q+r, cg*56 + s*14 + c2*7 + j] = y[ht*4+r, j, cg*32+8q+2s+c2]
    """
    y = np.empty((OH, OW, C), dtype=np.float32)
    v = raw.reshape(2, 128, NCG, 4, 2, OW)
    for ht in range(2):
        I = 4 if ht == 0 else 3
        yv = y[ht * 4 : ht * 4 + I].reshape(I, OW, NCG, 32)
        for q in range(4):
            blk = v[ht, 32 * q : 32 * q + I]  # [r, cg, s, c2, j]
            # y[ht*4+r, j, cg*32 + 8q + (2s+c2)]
            yv[:, :, :, 8 * q : 8 * q + 8] = blk.transpose(0, 4, 1, 2, 3).reshape(
                I, OW, NCG, 8
            )
    return y


def kernel(x, out_h=7, out_w=7, _trace=False, **_ignored):
    from concourse.bass_utils import run_bass_kernel_spmd

    x = np.asarray(x, dtype=np.float32)
    assert x.shape == (B, H, W, C), x.shape
    assert int(out_h) == OH and int(out_w) == OW

    if "nc" not in _CACHE:
        _CACHE["nc"] = _build()
    nc = _CACHE["nc"]

    mh = _mh_matrix()
    in_maps = [
        {
            # [H, W, C] -> [H, C, W] in bf16, flattened to [H, C*W]
            "x": np.ascontiguousarray(x[b].transpose(0, 2, 1))
            .astype(ml_dtypes.bfloat16)
            .reshape(H, CW),
            "mh": mh,
        }
        for b in range(B)
    ]
    res = run_bass_kernel_spmd(nc, in_maps, core_ids=list(range(B)), trace=_trace)
    _CACHE["last_res"] = res
    outs = [_unscramble(res.results[b]["out"]) for b in range(B)]
    return np.stack(outs, axis=0).astype(np.float32)


# revision 24
# speedup vs baseline: 1.1376x; 1.1376x over previous
"""Adaptive average pooling [8,224,224,256] -> [8,7,7,256] on 8 TRN2 NeuronCores.

Strategy: data-parallel over batch (1 sample per core, no collectives).
Pooling windows are exact 32x32 blocks (224/7 = 32). Each sample is
transposed host-side to [H, C, W] and cast to bf16 (halves HBM traffic;
window sums accumulate in fp32 so only input quantization enters).

Per core:
  - partition dim = image row h, free dim = flattened (c, w); every DMA is
    fully contiguous (14 KiB per partition per tile).
  - stage 1 (reduce over the 32 h rows of each window): TensorE matmuls
    against a block matrix of 1/1024, contracting the partition dim. Output
    chunks are packed into PSUM partition quarters (4 chunks x 448 per
    quarter) so downstream ops use ~full lanes.
  - stage 2 (reduce over the 32 w positions): VectorE strided reduce
    directly from PSUM (only 1/32 of the volume reaches VectorE).
  - result is DMA'd out in the raw packed layout; host numpy unscrambles
    the 180 KB output.
"""

import ml_dtypes
import numpy as np

B, H, W, C = 8, 224, 224, 256
OH, OW = 7, 7
WIN = H // OH  # 32
CW = C * W  # 57344 elements per row, layout (c, w)
CG = 32  # channels per input tile
CHUNK = CG * W  # 7168 elements = 14 KiB (bf16) per partition
NCG = C // CG  # 8 channel groups
CPAIR = 2 * W  # 448: matmul rhs chunk = 2 channels x 224 w
YF = 4 * 2 * OW  # 56 floats of ybuf per channel group

_CACHE = {}


def _build():
    import concourse.bass as bass
    import concourse.mybir as mybir
    from concourse import bacc, tile

    f32 = mybir.dt.float32
    bf16 = mybir.dt.bfloat16
    nc = bacc.Bacc(
        "TRN2",
        target_bir_lowering=False,
        debug=False,
        enable_asserts=False,
        num_devices=B,
    )
    x = nc.dram_tensor("x", [H, CW], bf16, kind="ExternalInput").ap()
    mh = nc.dram_tensor("mh", [128, 64], bf16, kind="ExternalInput").ap()
    out = nc.dram_tensor("out", [128, 2 * NCG * YF], f32, kind="ExternalOutput").ap()

    with tile.TileContext(nc) as tc:
        with (
            tc.tile_pool(name="consts", bufs=1) as cpool,
            tc.tile_pool(name="xin", bufs=6) as inpool,
            tc.tile_pool(name="ybuf", bufs=2) as ypool,
            tc.tile_pool(name="psum", bufs=2, space=bass.MemorySpace.PSUM) as ppool,
        ):
            mh_t = cpool.tile([128, 64], bf16)
            nc.sync.dma_start(mh_t[:], mh[:])
            ybuf = ypool.tile([128, 2 * NCG * YF], f32)
            for ht in range(2):  # rows 0..127 / rows 96..223 (128 each:
                # full-partition DMAs run ~2x the rate of 96-partition ones,
                # so re-reading rows 96..127 with zero weights is a net win)
                r0 = ht * 96
                for cg in range(NCG):
                    t = inpool.tile([128, CHUNK], bf16)
                    nc.sync.dma_start(
                        t[:, :],
                        x[r0 : r0 + 128, cg * CHUNK : (cg + 1) * CHUNK],
                    )
                    ps = ppool.tile([128, 2048], f32)
                    # chunk m = channel pair; quarter q=m//4 rows 32q.., slot m%4
                    for m in range(16):
                        q, s = divmod(m, 4)
                        nc.tensor.matmul(
                            ps[32 * q : 32 * q + 32, 512 * s : 512 * s + CPAIR],
                            mh_t[:, ht * 32 : ht * 32 + 32],
                            t[:, m * CPAIR : (m + 1) * CPAIR],
                            start=True,
                            stop=True,
                            tile_position=(0, 32 * q),
                        )
                    # reduce w (unit stride innermost) straight out of PSUM:
                    # in [100][(s:4,x512)][(c2:2,x224)][(j:7,x32)][(w:32,x1)]
                    inap = ps[:128, :].rearrange("p (s f) -> p s f", s=4, f=512)[
                        :, :, :CPAIR
                    ].rearrange("p s (c j w) -> p s c j w", c=2, j=OW, w=WIN)
                    yoff = ht * NCG * YF + cg * YF
                    outap = ybuf[:128, yoff : yoff + YF].rearrange(
                        "p (s c j) -> p s c j", s=4, c=2, j=OW
                    )
                    nc.vector.tensor_reduce(
                        out=outap,
                        in_=inap,
                        axis=mybir.AxisListType.X,
                        op=mybir.AluOpType.add,
                    )
            # single output DMA at the very end; by then all input triggers
            # have issued, so the sync-ring wait stalls nothing
            nc.sync.dma_start(out[:, :], ybuf[:, :])
    nc.compile()
    return nc


def _mh_matrix():
    # col block 0..31 (ht0, rows 0..127): col p//32 selects h-windows 0..3;
    # col block 32..63 (ht1, rows 96..223): rows 96..127 are re-read padding
    # with zero weight, rows 128..223 map to h-windows 4..6. Unused columns
    # stay zero so matmuls fill the whole PSUM quarter with defined zeros.
    m = np.zeros((128, 64), dtype=ml_dtypes.bfloat16)
    for p in range(128):
        m[p, p // WIN] = 1.0 / (WIN * WIN)  # ht0
        if p >= 32:
            m[p, 32 + p // 32 - 1] = 1.0 / (WIN * WIN)  # ht1
    return m


def _unscramble(raw):
    """raw [128, 2*448] packed -> y [7, 7, 256].

    raw[ht, 32q+r, cg*56 + s*14 + c2*7 + j] = y[ht*4+r, j, cg*32+8q+2s+c2]
    """
    y = np.empty((OH, OW, C), dtype=np.float32)
    v = raw.reshape(128, 2, NCG, 4, 2, OW).transpose(1, 0, 2, 3, 4, 5)
    for ht in range(2):
        I = 4 if ht == 0 else 3
        yv = y[ht * 4 : ht * 4 + I].reshape(I, OW, NCG, 32)
        for q in range(4):
            blk = v[ht, 32 * q : 32 * q + I]  # [r, cg, s, c2, j]
            # y[ht*4+r, j, cg*32 + 8q + (2s+c2)]
            yv[:, :, :, 8 * q : 8 * q + 8] = blk.transpose(0, 4, 1, 2, 3).reshape(
                I, OW, NCG, 8
            )
    return y


def kernel(x, out_h=7, out_w=7, _trace=False, **_ignored):
    from concourse.bass_utils import run_bass_kernel_spmd

    x = np.asarray(x, dtype=np.float32)
    assert x.shape == (B, H, W, C), x.shape
    assert int(out_h) == OH and int(out_w) == OW

    if "nc" not in _CACHE:
        _CACHE["nc"] = _build()
    nc = _CACHE["nc"]

    mh = _mh_matrix()
    in_maps = [
        {
            # [H, W, C] -> [H, C, W] in bf16, flattened to [H, C*W]
            "x": np.ascontiguousarray(x[b].transpose(0, 2, 1))
            .astype(ml_dtypes.bfloat16)
            .reshape(H, CW),
            "mh": mh,
        }
        for b in range(B)
    ]
    res = run_bass_kernel_spmd(nc, in_maps, core_ids=list(range(B)), trace=_trace)
    _CACHE["last_res"] = res
    outs = [_unscramble(res.results[b]["out"]) for b in range(B)]
    return np.stack(outs, axis=0).astype(np.float32)


# revision 26
# speedup vs baseline: 1.1623x; 1.0218x over previous
"""Adaptive average pooling [8,224,224,256] -> [8,7,7,256] on 8 TRN2 NeuronCores.

Strategy: data-parallel over batch (1 sample per core, no collectives).
Pooling windows are exact 32x32 blocks (224/7 = 32). Each sample is
transposed host-side to [H, C, W] and cast to bf16 (halves HBM traffic;
window sums accumulate in fp32 so only input quantization enters).

Per core:
  - partition dim = image row h, free dim = flattened (c, w); every DMA is
    fully contiguous (14 KiB per partition per tile).
  - stage 1 (reduce over the 32 h rows of each window): TensorE matmuls
    against a block matrix of 1/1024, contracting the partition dim. Output
    chunks are packed into PSUM partition quarters (4 chunks x 448 per
    quarter) so downstream ops use ~full lanes.
  - stage 2 (reduce over the 32 w positions): VectorE strided reduce
    directly from PSUM (only 1/32 of the volume reaches VectorE).
  - result is DMA'd out in the raw packed layout; host numpy unscrambles
    the 180 KB output.
"""

import ml_dtypes
import numpy as np

B, H, W, C = 8, 224, 224, 256
OH, OW = 7, 7
WIN = H // OH  # 32
CW = C * W  # 57344 elements per row, layout (c, w)
CG = 32  # channels per input tile
CHUNK = CG * W  # 7168 elements = 14 KiB (bf16) per partition
NCG = C // CG  # 8 channel groups
CPAIR = 2 * W  # 448: matmul rhs chunk = 2 channels x 224 w
YF = 4 * 2 * OW  # 56 floats of ybuf per channel group

_CACHE = {}


def _build():
    import concourse.bass as bass
    import concourse.mybir as mybir
    from concourse import bacc, tile

    f32 = mybir.dt.float32
    bf16 = mybir.dt.bfloat16
    nc = bacc.Bacc(
        "TRN2",
        target_bir_lowering=False,
        debug=False,
        enable_asserts=False,
        num_devices=B,
    )
    x = nc.dram_tensor("x", [H, CW], bf16, kind="ExternalInput").ap()
    mh = nc.dram_tensor("mh", [128, 64], bf16, kind="ExternalInput").ap()
    out = nc.dram_tensor("out", [128, 2 * NCG * YF], f32, kind="ExternalOutput").ap()

    with tile.TileContext(nc) as tc:
        with (
            tc.tile_pool(name="consts", bufs=1) as cpool,
            tc.tile_pool(name="xin", bufs=6) as inpool,
            tc.tile_pool(name="ybuf", bufs=2) as ypool,
            tc.tile_pool(name="psum", bufs=2, space=bass.MemorySpace.PSUM) as ppool,
        ):
            mh_t = cpool.tile([128, 64], bf16)
            # scalar ring: keeps the input queue head free for x tiles
            nc.scalar.dma_start(mh_t[:], mh[:])
            ybuf = ypool.tile([128, 2 * NCG * YF], f32)
            for ht in range(2):  # rows 0..127 / rows 96..223 (128 each:
                # full-partition DMAs run ~2x the rate of 96-partition ones,
                # so re-reading rows 96..127 with zero weights is a net win)
                r0 = ht * 96
                for cg in range(NCG):
                    t = inpool.tile([128, CHUNK], bf16)
                    nc.sync.dma_start(
                        t[:, :],
                        x[r0 : r0 + 128, cg * CHUNK : (cg + 1) * CHUNK],
                    )
                    ps = ppool.tile([128, 2048], f32)
                    # chunk m = channel pair; quarter q=m//4 rows 32q.., slot m%4
                    for m in range(16):
                        q, s = divmod(m, 4)
                        nc.tensor.matmul(
                            ps[32 * q : 32 * q + 32, 512 * s : 512 * s + CPAIR],
                            mh_t[:, ht * 32 : ht * 32 + 32],
                            t[:, m * CPAIR : (m + 1) * CPAIR],
                            start=True,
                            stop=True,
                            tile_position=(0, 32 * q),
                        )
                    # reduce w (unit stride innermost) straight out of PSUM:
                    # in [128][(s:4,x512)][(c2:2,x224)][(j:7,x32)][(w:32,x1)]
                    inap = ps[:128, :].rearrange("p (s f) -> p s f", s=4, f=512)[
                        :, :, :CPAIR
                    ].rearrange("p s (c j w) -> p s c j w", c=2, j=OW, w=WIN)
                    yoff = ht * NCG * YF + cg * YF
                    outap = ybuf[:128, yoff : yoff + YF].rearrange(
                        "p (s c j) -> p s c j", s=4, c=2, j=OW
                    )
                    nc.vector.tensor_reduce(
                        out=outap,
                        in_=inap,
                        axis=mybir.AxisListType.X,
                        op=mybir.AluOpType.add,
                    )
            # single output DMA at the very end; by then all input triggers
            # have issued, so the sync-ring wait stalls nothing
            nc.sync.dma_start(out[:, :], ybuf[:, :])
    nc.compile()
    return nc


def _mh_matrix():
    # col block 0..31 (ht0, rows 0..127): col p//32 selects h-windows 0..3;
    # col block 32..63 (ht1, rows 96..223): rows 96..127 are re-read padding
    # with zero weight, rows 128..223 map to h-windows 4..6. Unused columns
    # stay zero so matmuls fill the whole PSUM quarter with defined zeros.
    m = np.zeros((128, 64), dtype=ml_dtypes.bfloat16)
    for p in range(128):
        m[p, p // WIN] = 1.0 / (WIN * WIN)  # ht0
        if p >= 32:
            m[p, 32 + p // 32 - 1] = 1.0 / (WIN * WIN)  # ht1
    return m


def _unscramble(raw):
    """raw [128, 2*448] packed -> y [7, 7, 256].

    raw[ht, 32q+r, cg*56 + s*14 + c2*7 + j] = y[ht*4+r, j, cg*32+8q+2s+c2]
    """
    y = np.empty((OH, OW, C), dtype=np.float32)
    v = raw.reshape(128, 2, NCG, 4, 2, OW).transpose(1, 0, 2, 3, 4, 5)
    for ht in range(2):
        I = 4 if ht == 0 else 3
        yv = y[ht * 4 : ht * 4 + I].reshape(I, OW, NCG, 32)
        for q in range(4):
            blk = v[ht, 32 * q : 32 * q + I]  # [r, cg, s, c2, j]
            # y[ht*4+r, j, cg*32 + 8q + (2s+c2)]
            yv[:, :, :, 8 * q : 8 * q + 8] = blk.transpose(0, 4, 1, 2, 3).reshape(
                I, OW, NCG, 8
            )
    return y


def kernel(x, out_h=7, out_w=7, _trace=False, **_ignored):
    from concourse.bass_utils import run_bass_kernel_spmd

    x = np.asarray(x, dtype=np.float32)
    assert x.shape == (B, H, W, C), x.shape
    assert int(out_h) == OH and int(out_w) == OW

    if "nc" not in _CACHE:
        _CACHE["nc"] = _build()
    nc = _CACHE["nc"]

    mh = _mh_matrix()
    in_maps = [
        {
            # [H, W, C] -> [H, C, W] in bf16, flattened to [H, C*W]
            "x": np.ascontiguousarray(x[b].transpose(0, 2, 1))
            .astype(ml_dtypes.bfloat16)
            .reshape(H, CW),
            "mh": mh,
        }
        for b in range(B)
    ]
    res = run_bass_kernel_spmd(nc, in_maps, core_ids=list(range(B)), trace=_trace)
    _CACHE["last_res"] = res
    outs = [_unscramble(res.results[b]["out"]) for b in range(B)]
    return np.stack(outs, axis=0).astype(np.float32)
